# revision 45
# baseline (speedup 1.0000x reference)
"""Trainium2 Bass kernel for nn_Mixture_Loss_74053826118054.

Strategy (valid-row compaction + data parallel over 8 cores):
  Every term of the loss depends only on 5 per-(s,b)-row reductions over D:
    ll = sum_d l^2,  tt = sum_d t^2,  lt = sum_d l*t,
    ln = sum_d l[s]*l[s+1]  (consecutive sentences, same batch),
    tn = sum_d t[s]*t[s+1]
  The padding mask is known on the host, and every quantity is only ever
  USED on valid rows (~50%): masked MSE / cos / deltas all multiply by the
  valid mask, and ln/tn are only used on valid PAIRS (both rows valid,
  ~25%). The host packs only the valid rows, batch-major with maximal
  consecutive-valid runs kept contiguous, and ships the packed array:
  half the HBM traffic and half the compute of the dense kernel.

  Rows in runs of length >= 2 ("run region") need all five quantities;
  isolated valid rows ("iso region", no valid neighbor) need only
  ll/tt/lt. The packed list is [run rows | iso rows]; overflow iso rows
  spill into the run region's spare slots (their pair products are
  garbage the host ignores). Pair validity is re-derived on the host
  from the packed (b, s) list, so any garbage pair (run boundaries,
  spilled iso rows, zero padding) is dropped exactly.

Device layout per core: partitions hold RG consecutive packed rows
(+1 overlap slot = next partition's/core's first row, so consecutive-row
products are free-axis slices) plus IG iso rows. The host builds the
per-core input SLOT-MAJOR ([slot, partition, {l,t}, D] — exactly the
SBUF layout), so every chunk load is one contiguous 1 MB DMA with one
8 KB descriptor per partition (~400 GB/s observed vs ~275 GB/s for the
strided layout). Per chunk: ACT does the two squares with fused
accumulate; DVE does lt/ln/tn as scalar_tensor_tensor with fused
accumulate. No GpSimd: its SBUF port is shared with DVE ("POOL slot")
and concurrent GpSimd ops double DVE 2-port op latency (measured on the
dense baseline: stt 2142 ns avg with GpSimd active vs 1135 ns clean).

DMA schedule: all chunk loads go back-to-back on the Sync engine's
HWDGE ring in consumption order, overlap slot last — a single ring
drains strictly FIFO, so each chunk gets the full wire and completes
in order (splitting across the two HWDGE rings measured ~2 us slower:
packet-granularity arbitration delays the early chunks). One semaphore
per chunk — completions are only FIFO per SDMA engine, so a summed
counter can be satisfied by a MIX of chunks (a race observed as
intermittent garbage). Each compute engine stores its own merged
result block (ACT: [ll|tt] on its ring right after its drain; DVE's
[lt|ln|tn] via Sync after dve_done), so the only cross-engine sync is
dve_done and the final out_sem wait. The tiny O(S*B) finish (cos,
deltas, rank-compaction, delta-of-delta) runs on host in float64,
reproducing the reference semantics exactly.

Timing on the fixed-seed data (RG=6, IG=2, 9 MB/core): ~40-43 us vs
91.5 us for the dense 3-engine baseline. Breakdown: ~7 us NEFF
preamble (start barrier + instruction load, run-to-run jitter +-2 us),
~6 us first-two-chunk fill, 21.8 us dense DVE (20 stt ops, the
compute-side floor: 3 x 1146 ns per run chunk + 1 per iso chunk),
~3.5 us store/epilogue tail. The wire itself streams at ~2.5 us/MB
overlapped under DVE. f16/bf16 data is NOT usable: the delta-of-delta
term amplifies input quantization into a ~25x loss error (measured);
all five reductions must be f32 exact.
"""

import numpy as np

from contextlib import ExitStack

import concourse.bass as bass
import concourse.mybir as mybir
from concourse.bass_utils import run_bass_kernel_spmd

F32 = mybir.dt.float32
AF = mybir.ActivationFunctionType
ALU = mybir.AluOpType

N_CORES = 8
S, B, D = 64, 256, 1024
P = 128
DUAL_RING = False       # measured slower: ring arbitration delays early chunks

_cached = {}


def _build_program(RG, IG):
    """Bass program for RG run-chunks (+1 overlap slot) and IG iso-chunks."""
    key = (RG, IG)
    if key in _cached:
        return _cached[key]
    NSLOT = RG + IG + 1          # run slots + iso slots + overlap (last)
    OV = RG + IG                 # overlap slot index
    NC = RG + IG                 # result columns for ll/tt/lt
    nc = bass.Bass()
    # slot-major, exactly the SBUF layout: x[slot, partition] = [l | t]
    # of that slot's row -> every chunk load is one contiguous 1MB region
    x_in = nc.dram_tensor("x", [NSLOT, P, 2, D], F32, kind="ExternalInput")
    # merged outputs: one store per compute engine
    out_act = nc.dram_tensor("act", [P, 2 * NC], F32, kind="ExternalOutput")
    out_dve = nc.dram_tensor("dve", [P, NC + 2 * RG], F32,
                             kind="ExternalOutput")

    with ExitStack() as stack:
        ec = stack.enter_context
        # one semaphore per chunk: DMA completions are only FIFO per SDMA
        # engine, so a summed counter can pass with a mix of chunks
        csem = [ec(nc.semaphore(f"c{i}")) for i in range(NSLOT)]
        out_sem = ec(nc.semaphore("out"))
        dve_done = ec(nc.semaphore("dvedone"))
        xbig = ec(nc.sbuf_tensor([P, NSLOT * 2 * D], F32))
        dummies = ec(nc.sbuf_tensor([P, 8], F32))
        ract = ec(nc.sbuf_tensor([P, 2 * NC], F32))       # [ll | tt]
        rdve = ec(nc.sbuf_tensor([P, NC + 2 * RG], F32))  # [lt | ln | tn]
        block = ec(nc.Block())
        xc = xbig.ap().rearrange("p (c v d) -> p c v d", v=2, d=D)

        def chunk(slot, half):
            return xc[:, slot, half, :]

        def bcast(k):
            return dummies.ap()[:, k:k + 1].broadcast_to((P, D))

        rll = ract.ap()[:, 0:NC]
        rtt = ract.ap()[:, NC:2 * NC]
        rlt = rdve.ap()[:, 0:NC]
        rln = rdve.ap()[:, NC:NC + RG]
        rtn = rdve.ap()[:, NC + RG:NC + 2 * RG]

        @block.sync
        def _(sync):
            # all loads on this one HWDGE ring, consumption order, overlap
            # slot last: a single ring drains strictly FIFO at full wire
            for s in range(NSLOT):
                if s % 2 == 1 and DUAL_RING:
                    continue             # odd slots ride the ACT ring
                sync.dma_start(out=xc[:, s, :, :],
                               in_=x_in[s]).then_inc(csem[s], 16)
            sync.wait_ge(dve_done, 1)
            sync.dma_start(out=out_dve[:], in_=rdve.ap()).then_inc(
                out_sem, 16)
            sync.wait_ge(out_sem, 32)

        @block.scalar
        def _(scalar):
            if DUAL_RING:
                for s in range(1, NSLOT, 2):
                    scalar.dma_start(out=xc[:, s, :, :],
                                     in_=x_in[s]).then_inc(csem[s], 16)
            for col in range(NC):        # run slots then iso slots = cols
                scalar.wait_ge(csem[col], 16)
                scalar.activation(bcast(0), chunk(col, 0), AF.Square,
                                  accum_out=rll[:, col:col + 1])
                scalar.activation(bcast(1), chunk(col, 1), AF.Square,
                                  accum_out=rtt[:, col:col + 1])
            scalar.drain()
            scalar.dma_start(out=out_act[:], in_=ract.ap()).then_inc(
                out_sem, 16)

        @block.vector
        def _(vector):
            for j in range(RG):
                vector.wait_ge(csem[j], 16)
                vector.scalar_tensor_tensor(
                    out=bcast(2), in0=chunk(j, 0), scalar=0.0,
                    in1=chunk(j, 1), op0=ALU.bypass, op1=ALU.mult,
                    accum_out=rlt[:, j:j + 1])
                if j < RG - 1:           # j = RG-1 pairs with the overlap
                    vector.wait_ge(csem[j + 1], 16)   # slot, loaded last
                    vector.scalar_tensor_tensor(
                        out=bcast(3), in0=chunk(j, 0), scalar=0.0,
                        in1=chunk(j + 1, 0), op0=ALU.bypass, op1=ALU.mult,
                        accum_out=rln[:, j:j + 1])
                    vector.scalar_tensor_tensor(
                        out=bcast(4), in0=chunk(j, 1), scalar=0.0,
                        in1=chunk(j + 1, 1), op0=ALU.bypass, op1=ALU.mult,
                        accum_out=rtn[:, j:j + 1])
            for k in range(IG):
                slot = RG + k
                vector.wait_ge(csem[slot], 16)
                vector.scalar_tensor_tensor(
                    out=bcast(2), in0=chunk(slot, 0), scalar=0.0,
                    in1=chunk(slot, 1), op0=ALU.bypass, op1=ALU.mult,
                    accum_out=rlt[:, slot:slot + 1])
            vector.wait_ge(csem[OV], 16)
            vector.scalar_tensor_tensor(
                out=bcast(3), in0=chunk(RG - 1, 0), scalar=0.0,
                in1=chunk(OV, 0), op0=ALU.bypass, op1=ALU.mult,
                accum_out=rln[:, RG - 1:RG])
            vector.scalar_tensor_tensor(
                out=bcast(4), in0=chunk(RG - 1, 1), scalar=0.0,
                in1=chunk(OV, 1), op0=ALU.bypass, op1=ALU.mult,
                accum_out=rtn[:, RG - 1:RG])
            vector.drain().then_inc(dve_done, 1)

    _cached[key] = nc
    return nc


def _plan(mask):
    """Packed-row plan from the padding mask.

    Returns (bs_b, bs_s, pair_ok, RG, IG): packed order = all rows of
    runs (length >= 2, batch-major, runs contiguous) then isolated valid
    rows. pair_ok[g] marks packed-adjacent pairs (g, g+1) that are true
    consecutive same-batch valid pairs.
    """
    valid = ~mask                                   # (B, S)
    left = np.zeros_like(valid)
    left[:, 1:] = valid[:, :-1]
    right = np.zeros_like(valid)
    right[:, :-1] = valid[:, 1:]
    iso = valid & ~left & ~right
    runm = valid & ~iso
    rb, rs = np.nonzero(runm)                       # lexicographic: runs stay
    ib, is_ = np.nonzero(iso)                       # contiguous in order
    bs_b = np.concatenate([rb, ib])
    bs_s = np.concatenate([rs, is_])
    n_run = len(rb)
    tot = len(bs_b)
    pair_ok = (bs_b[:-1] == bs_b[1:]) & (bs_s[:-1] + 1 == bs_s[1:])
    RG = max(1, -(-n_run // (N_CORES * P)))
    IG = max(0, -(-(tot - N_CORES * P * RG) // (N_CORES * P)))
    return bs_b, bs_s, pair_ok, RG, IG


def _run_device(logits, tgt_out, plan, trace=False):
    bs_b, bs_s, pair_ok, RG, IG = plan
    nc = _build_program(RG, IG)
    NC = RG + IG
    tot = len(bs_b)

    lf = np.swapaxes(logits, 0, 1)                  # (B, S, D) view
    tf = np.swapaxes(tgt_out, 0, 1)
    # packed[r] = [l_r | t_r] interleaved
    packed = np.zeros((N_CORES * P * NC + 1, 2, D), np.float32)
    packed[:tot, 0] = lf[bs_b, bs_s]
    packed[:tot, 1] = tf[bs_b, bs_s]

    iso_base = N_CORES * P * RG
    in_maps = []
    for c in range(N_CORES):
        x = np.empty((RG + IG + 1, P, 2, D), np.float32)
        run = packed[c * P * RG:(c + 1) * P * RG].reshape(P, RG, 2, D)
        x[:RG] = run.transpose(1, 0, 2, 3)          # slot-major
        if IG:
            iso = packed[iso_base + c * P * IG:
                         iso_base + (c + 1) * P * IG].reshape(P, IG, 2, D)
            x[RG:RG + IG] = iso.transpose(1, 0, 2, 3)
        # overlap slot: next partition's (or next core's) first run row
        x[RG + IG] = packed[c * P * RG + RG:
                            (c + 1) * P * RG + RG:RG]
        in_maps.append({"x": x})
    kres = run_bass_kernel_spmd(nc, in_maps, list(range(N_CORES)),
                                trace=trace)

    # device columns [run slots | iso slots] -> packed positions
    full = {}
    res = kres.results
    for q, grab in (("ll", lambda a: a["act"][:, :NC]),
                    ("tt", lambda a: a["act"][:, NC:]),
                    ("lt", lambda a: a["dve"][:, :NC])):
        runp = np.concatenate([grab(res[c])[:, :RG].reshape(P * RG)
                               for c in range(N_CORES)])
        isop = (np.concatenate([grab(res[c])[:, RG:].reshape(P * IG)
                                for c in range(N_CORES)]) if IG else
                np.zeros(0, np.float32))
        full[q] = np.concatenate([runp, isop])
    for q, sl in (("ln", slice(NC, NC + RG)), ("tn", slice(NC + RG, None))):
        full[q] = np.concatenate([res[c]["dve"][:, sl].reshape(P * RG)
                                  for c in range(N_CORES)])
    return full, kres


def _finish_host(rows, mask):
    """Host-side float64 finish: reproduce reference semantics exactly."""
    ll = rows["ll"].astype(np.float64)
    tt = rows["tt"].astype(np.float64)
    lt = rows["lt"].astype(np.float64)
    ln = rows["ln"].astype(np.float64)
    tn = rows["tn"].astype(np.float64)

    valid = ~mask                     # (B, S)
    n_valid = float(valid.sum())

    # masked MSE: sum over valid rows of sum_d (l-t)^2 = ll - 2lt + tt
    mse = ((ll - 2.0 * lt + tt) * valid).sum() / (n_valid * D)

    # CosineEmbeddingLoss part (eps = 1e-8)
    na = np.maximum(np.sqrt(ll), 1e-8)
    nb = np.maximum(np.sqrt(tt), 1e-8)
    c = lt / (na * nb)
    loss_cos = ((1.0 - c) * valid).sum() / n_valid

    # consecutive-sentence cosine deltas (eps = 1e-6), shape (B, S-1)
    nl = np.maximum(np.sqrt(ll), 1e-6)
    nt = np.maximum(np.sqrt(tt), 1e-6)
    d_l = ln[:, :S - 1] / (nl[:, :-1] * nl[:, 1:])
    d_t = tn[:, :S - 1] / (nt[:, :-1] * nt[:, 1:])
    pair_valid = valid[:, :-1] & valid[:, 1:]
    cnt = int(pair_valid.sum())
    loss_delta = (np.square(d_l - d_t) * pair_valid).sum() / max(cnt, 1)

    # delta-of-delta on the compacted (valid-only, batch-major) delta lists
    L = B * (S - 1)
    pvf = pair_valid.reshape(-1)

    def dd(d_flat):
        dense = np.zeros(L, np.float64)
        dense[:cnt] = d_flat[pvf]
        prev = dense[:-1]
        den = np.where(prev != 0, prev, 1e-6)
        return (dense[1:] - prev) / den

    dd_l = dd(d_l.reshape(-1))
    dd_t = dd(d_t.reshape(-1))
    dd_valid = np.arange(L - 1) < (cnt - 1)
    n_dd = float(max(cnt - 1, 1))
    loss_dd = (np.square(dd_l - dd_t) * dd_valid).sum() / n_dd / 100.0

    return mse + loss_cos + loss_delta + loss_dd


def kernel(logits, tgt_out, tgt_padding_mask, _trace=False):
    logits = np.asarray(logits, dtype=np.float32)
    tgt_out = np.asarray(tgt_out, dtype=np.float32)
    mask = np.asarray(tgt_padding_mask).astype(bool)

    plan = _plan(mask)
    bs_b, bs_s, pair_ok, RG, IG = plan
    tot = len(bs_b)
    packed, kres = _run_device(logits, tgt_out, plan, trace=_trace)

    # scatter packed results back to full (B, S) arrays; untouched
    # positions stay 0 and are masked out in the finish.
    rows = {}
    for q in ("ll", "tt", "lt"):
        f = np.zeros((B, S), np.float32)
        f[bs_b, bs_s] = packed[q][:tot]
        rows[q] = f
    gok = np.flatnonzero(pair_ok)     # all true pairs live in the run region
    for q in ("ln", "tn"):
        f = np.zeros((B, S), np.float32)
        f[bs_b[gok], bs_s[gok]] = packed[q][gok]
        rows[q] = f

    total = _finish_host(rows, mask)
    out = np.array(total, dtype=np.float32)
    if _trace:
        return out, kres
    return out


# revision 46
# speedup vs baseline: 1.1505x; 1.1505x over previous
"""Trainium2 Bass kernel for nn_Mixture_Loss_74053826118054.

Strategy (valid-row compaction + data parallel over 8 cores):
  Every term of the loss depends only on 5 per-(s,b)-row reductions over D:
    ll = sum_d l^2,  tt = sum_d t^2,  lt = sum_d l*t,
    ln = sum_d l[s]*l[s+1]  (consecutive sentences, same batch),
    tn = sum_d t[s]*t[s+1]
  The padding mask is known on the host, and every quantity is only ever
  USED on valid rows (~50%): masked MSE / cos / deltas all multiply by the
  valid mask, and ln/tn are only used on valid PAIRS (both rows valid,
  ~25%). The host packs only the valid rows and ships the packed array:
  half the HBM traffic and half the compute of the dense kernel.

  Rows are split into three regions by how much pair work they need:
  - "long region": rows of runs >= 3 consecutive valid, runs contiguous,
    chained across partitions with a +1 overlap slot; ln/tn computed for
    every slot adjacency (garbage at run boundaries, dropped on host).
  - "pair region": rows of length-2 runs, stored pair-aligned (first row
    in an even slot, second in the odd slot beside it). One ln/tn op per
    PAIR instead of per row — half the DVE pair work for these rows.
  - "iso region": isolated valid rows (no valid neighbor): ll/tt/lt only.
  Overflow rows spill into other regions' padding holes; pair validity
  is re-derived on the host from the position-aligned (b, s) map, so any
  garbage product (run boundaries, spilled rows, padding) is dropped
  exactly.

Device layout per core: the host builds the input SLOT-MAJOR
([slot, partition, {l,t}, D] - exactly the SBUF layout), so every chunk
load is one contiguous 1 MB DMA with one 8 KB descriptor per partition
(~400 GB/s observed vs ~275 GB/s for a strided layout). Per chunk: ACT
does the two squares with fused accumulate; DVE does lt/ln/tn as
scalar_tensor_tensor with fused accumulate. No GpSimd: its SBUF port is
shared with DVE ("POOL slot") and concurrent GpSimd ops double DVE
2-port op latency (measured: stt 2142 ns avg with GpSimd active vs
1135 ns clean). f16/bf16 data is NOT usable: the delta-of-delta term
amplifies input quantization into a ~25x loss error (measured); all
five reductions must be f32 exact.

DMA schedule: all chunk loads go back-to-back on the Sync engine's
HWDGE ring in consumption order, overlap slot last - a single ring
drains strictly FIFO, so each chunk gets the full wire and completes in
order (splitting across the two HWDGE rings measured ~2 us slower:
packet-granularity arbitration delays the early chunks). One semaphore
per chunk - completions are only FIFO per SDMA engine, so a summed
counter can be satisfied by a MIX of chunks (a race observed as
intermittent garbage). Each compute engine stores its own merged result
block (ACT: [ll|tt] on its ring right after its drain; DVE's [lt|ln|tn]
via Sync after dve_done). The tiny O(S*B) finish (cos, deltas,
rank-compaction, delta-of-delta) runs on host in float64, reproducing
the reference semantics exactly.

Timing on the fixed-seed data (RL=4, K2=1, IG=2, 9 MB/core): ~7 us NEFF
preamble (start barrier + instruction load, +-2 us run-to-run jitter),
~6 us first-two-chunk fill, ~20.6 us dense DVE (18 stt ops - the
compute floor; ACT is ~19.7 us), ~3.5 us store/epilogue tail. The wire
streams at ~2.5 us/MB fully overlapped under DVE.
"""

import numpy as np

from contextlib import ExitStack

import concourse.bass as bass
import concourse.mybir as mybir
from concourse.bass_utils import run_bass_kernel_spmd

F32 = mybir.dt.float32
AF = mybir.ActivationFunctionType
ALU = mybir.AluOpType

N_CORES = 8
S, B, D = 64, 256, 1024
P = 128

_cached = {}


def _build_program(RL, K2, IG):
    """RL chained long-run slots, K2 aligned pairs (2*K2 slots), IG iso
    slots, +1 overlap slot (loaded last)."""
    key = (RL, K2, IG)
    if key in _cached:
        return _cached[key]
    NC = RL + 2 * K2 + IG        # content slots = result columns
    NSLOT = NC + 1
    OV = NC                      # overlap slot index
    NLN = RL + K2                # ln/tn result columns
    nc = bass.Bass()
    # slot-major, exactly the SBUF layout: x[slot, partition] = [l | t]
    x_in = nc.dram_tensor("x", [NSLOT, P, 2, D], F32, kind="ExternalInput")
    out_act = nc.dram_tensor("act", [P, 2 * NC], F32, kind="ExternalOutput")
    out_dve = nc.dram_tensor("dve", [P, NC + 2 * NLN], F32,
                             kind="ExternalOutput")

    with ExitStack() as stack:
        ec = stack.enter_context
        # one semaphore per chunk: DMA completions are only FIFO per SDMA
        # engine, so a summed counter can pass with a mix of chunks
        csem = [ec(nc.semaphore(f"c{i}")) for i in range(NSLOT)]
        out_sem = ec(nc.semaphore("out"))
        dve_done = ec(nc.semaphore("dvedone"))
        xbig = ec(nc.sbuf_tensor([P, NSLOT * 2 * D], F32))
        dummies = ec(nc.sbuf_tensor([P, 8], F32))
        ract = ec(nc.sbuf_tensor([P, 2 * NC], F32))       # [ll | tt]
        rdve = ec(nc.sbuf_tensor([P, NC + 2 * NLN], F32))  # [lt | ln | tn]
        block = ec(nc.Block())
        xc = xbig.ap().rearrange("p (c v d) -> p c v d", v=2, d=D)

        def chunk(slot, half):
            return xc[:, slot, half, :]

        def bcast(k):
            return dummies.ap()[:, k:k + 1].broadcast_to((P, D))

        rll = ract.ap()[:, 0:NC]
        rtt = ract.ap()[:, NC:2 * NC]
        rlt = rdve.ap()[:, 0:NC]
        rln = rdve.ap()[:, NC:NC + NLN]
        rtn = rdve.ap()[:, NC + NLN:NC + 2 * NLN]

        @block.sync
        def _(sync):
            # all loads on this one HWDGE ring, consumption order, overlap
            # slot last: a single ring drains strictly FIFO at full wire
            for s in range(NSLOT):
                sync.dma_start(out=xc[:, s, :, :],
                               in_=x_in[s]).then_inc(csem[s], 16)
            sync.wait_ge(dve_done, 1)
            sync.dma_start(out=out_dve[:], in_=rdve.ap()).then_inc(
                out_sem, 16)
            sync.wait_ge(out_sem, 32)

        @block.scalar
        def _(scalar):
            for col in range(NC):        # every content slot: two squares
                scalar.wait_ge(csem[col], 16)
                scalar.activation(bcast(0), chunk(col, 0), AF.Square,
                                  accum_out=rll[:, col:col + 1])
                scalar.activation(bcast(1), chunk(col, 1), AF.Square,
                                  accum_out=rtt[:, col:col + 1])
            scalar.drain()
            scalar.dma_start(out=out_act[:], in_=ract.ap()).then_inc(
                out_sem, 16)

        @block.vector
        def _(vector):

            def stt(i0, i1, acc):
                vector.scalar_tensor_tensor(
                    out=bcast(2), in0=i0, scalar=0.0, in1=i1,
                    op0=ALU.bypass, op1=ALU.mult, accum_out=acc)

            for j in range(RL):          # chained long-run slots
                vector.wait_ge(csem[j], 16)
                stt(chunk(j, 0), chunk(j, 1), rlt[:, j:j + 1])
                if j < RL - 1:           # j = RL-1 pairs with the overlap
                    vector.wait_ge(csem[j + 1], 16)   # slot, loaded last
                    stt(chunk(j, 0), chunk(j + 1, 0), rln[:, j:j + 1])
                    stt(chunk(j, 1), chunk(j + 1, 1), rtn[:, j:j + 1])
            for u in range(K2):          # aligned len-2 pairs
                sa, sb = RL + 2 * u, RL + 2 * u + 1
                vector.wait_ge(csem[sa], 16)
                stt(chunk(sa, 0), chunk(sa, 1), rlt[:, sa:sa + 1])
                vector.wait_ge(csem[sb], 16)
                stt(chunk(sb, 0), chunk(sb, 1), rlt[:, sb:sb + 1])
                stt(chunk(sa, 0), chunk(sb, 0), rln[:, RL + u:RL + u + 1])
                stt(chunk(sa, 1), chunk(sb, 1), rtn[:, RL + u:RL + u + 1])
            for k in range(IG):          # iso slots: lt only
                slot = RL + 2 * K2 + k
                vector.wait_ge(csem[slot], 16)
                stt(chunk(slot, 0), chunk(slot, 1), rlt[:, slot:slot + 1])
            vector.wait_ge(csem[OV], 16)
            stt(chunk(RL - 1, 0), chunk(OV, 0), rln[:, RL - 1:RL])
            stt(chunk(RL - 1, 1), chunk(OV, 1), rtn[:, RL - 1:RL])
            vector.drain().then_inc(dve_done, 1)

    _cached[key] = nc
    return nc


def _plan(mask):
    """Position-aligned packing plan from the padding mask.

    Positions: [long region | pair region | iso region], each a multiple
    of N_CORES*P. Returns (pos_b, pos_s, RL, K2, IG) with pos_b == -1 at
    padding. Long-run rows sit at positions [0, n_long) with runs
    contiguous; length-2 runs at pair positions PB+2q (first row) and
    PB+2q+1 (second); iso rows fill the iso region then any padding
    holes (their products are garbage the host drops via pair_ok).
    """
    valid = ~mask                                   # (B, S)
    vp = np.zeros((B, S + 3), bool)
    vp[:, 1:S + 1] = valid
    v, left = vp[:, 1:S + 1], vp[:, 0:S]
    r1, r2 = vp[:, 2:S + 2], vp[:, 3:S + 3]
    iso = v & ~left & ~r1
    start2 = v & ~left & r1 & ~r2                   # first row of len-2 run
    second2 = np.zeros_like(v)
    second2[:, 1:] = start2[:, :-1]
    longm = v & ~iso & ~start2 & ~second2           # rows of runs >= 3

    lb, ls = np.nonzero(longm)                      # lexicographic: runs
    fb, fs = np.nonzero(start2)                     # stay contiguous
    ib, is_ = np.nonzero(iso)
    n_long, n2, n_iso = len(lb), len(fb), len(ib)
    tot = n_long + 2 * n2 + n_iso

    CP = N_CORES * P
    RL = max(1, -(-n_long // CP))
    K2 = -(-n2 // CP)
    IG = max(0, -(-(tot - CP * RL - CP * 2 * K2) // CP))
    PB = CP * RL                                    # pair region base
    IB = PB + CP * 2 * K2                           # iso region base
    NPOS = IB + CP * IG

    pos_b = np.full(NPOS, -1, np.int64)
    pos_s = np.zeros(NPOS, np.int64)
    pos_b[:n_long] = lb
    pos_s[:n_long] = ls
    if n2:
        pos_b[PB:PB + 2 * n2:2] = fb
        pos_s[PB:PB + 2 * n2:2] = fs
        pos_b[PB + 1:PB + 2 * n2:2] = fb
        pos_s[PB + 1:PB + 2 * n2:2] = fs + 1
    # iso rows: iso region first, then spill into padding holes
    holes = np.concatenate([np.arange(IB, NPOS),
                            np.arange(n_long, PB),
                            np.arange(PB + 2 * n2, IB)])
    pos_b[holes[:n_iso]] = ib
    pos_s[holes[:n_iso]] = is_
    return pos_b, pos_s, RL, K2, IG


def _run_device(logits, tgt_out, plan, trace=False):
    pos_b, pos_s, RL, K2, IG = plan
    nc = _build_program(RL, K2, IG)
    NC = RL + 2 * K2 + IG
    NLN = RL + K2
    NPOS = len(pos_b)
    PB = N_CORES * P * RL
    IB = PB + N_CORES * P * 2 * K2

    lf = np.swapaxes(logits, 0, 1)                  # (B, S, D) view
    tf = np.swapaxes(tgt_out, 0, 1)
    data = np.zeros((NPOS + 1, 2, D), np.float32)   # position-ordered rows
    filled = pos_b >= 0
    data[:NPOS][filled, 0] = lf[pos_b[filled], pos_s[filled]]
    data[:NPOS][filled, 1] = tf[pos_b[filled], pos_s[filled]]

    in_maps = []
    for c in range(N_CORES):
        x = np.empty((NC + 1, P, 2, D), np.float32)
        for base, width, lo in ((0, RL, 0), (PB, 2 * K2, RL),
                                (IB, IG, RL + 2 * K2)):
            if width == 0:
                continue
            blk = data[base + c * P * width:base + (c + 1) * P * width]
            x[lo:lo + width] = blk.reshape(P, width, 2, D).transpose(
                1, 0, 2, 3)
        # overlap slot: next partition's (or next core's) first long row
        x[NC] = data[c * P * RL + RL:(c + 1) * P * RL + RL:RL]
        in_maps.append({"x": x})
    kres = run_bass_kernel_spmd(nc, in_maps, list(range(N_CORES)),
                                trace=trace)

    # device columns -> position-ordered arrays [long | pair | iso]
    res = kres.results
    by_pos = {}
    for q, grab in (("ll", lambda a: a["act"][:, :NC]),
                    ("tt", lambda a: a["act"][:, NC:]),
                    ("lt", lambda a: a["dve"][:, :NC])):
        parts = []
        for lo, width in ((0, RL), (RL, 2 * K2), (RL + 2 * K2, IG)):
            if width:
                parts.append(np.concatenate(
                    [grab(res[c])[:, lo:lo + width].reshape(P * width)
                     for c in range(N_CORES)]))
        by_pos[q] = np.concatenate(parts)
    # ln/tn: cols [0, RL) = long adjacencies, cols [RL, RL+K2) = pairs
    for q, off in (("ln", NC), ("tn", NC + NLN)):
        lng = np.concatenate(
            [res[c]["dve"][:, off:off + RL].reshape(P * RL)
             for c in range(N_CORES)])
        f = np.zeros(NPOS, np.float32)
        f[:PB] = lng                    # value at pos g = product(g, g+1)
        if K2:
            pair = np.concatenate(
                [res[c]["dve"][:, off + RL:off + NLN].reshape(P * K2)
                 for c in range(N_CORES)])
            f[PB:IB:2] = pair
        by_pos[q] = f
    return by_pos, kres


def _finish_host(rows, mask):
    """Host-side float64 finish: reproduce reference semantics exactly."""
    ll = rows["ll"].astype(np.float64)
    tt = rows["tt"].astype(np.float64)
    lt = rows["lt"].astype(np.float64)
    ln = rows["ln"].astype(np.float64)
    tn = rows["tn"].astype(np.float64)

    valid = ~mask                     # (B, S)
    n_valid = float(valid.sum())

    # masked MSE: sum over valid rows of sum_d (l-t)^2 = ll - 2lt + tt
    mse = ((ll - 2.0 * lt + tt) * valid).sum() / (n_valid * D)

    # CosineEmbeddingLoss part (eps = 1e-8)
    na = np.maximum(np.sqrt(ll), 1e-8)
    nb = np.maximum(np.sqrt(tt), 1e-8)
    c = lt / (na * nb)
    loss_cos = ((1.0 - c) * valid).sum() / n_valid

    # consecutive-sentence cosine deltas (eps = 1e-6), shape (B, S-1)
    nl = np.maximum(np.sqrt(ll), 1e-6)
    nt = np.maximum(np.sqrt(tt), 1e-6)
    d_l = ln[:, :S - 1] / (nl[:, :-1] * nl[:, 1:])
    d_t = tn[:, :S - 1] / (nt[:, :-1] * nt[:, 1:])
    pair_valid = valid[:, :-1] & valid[:, 1:]
    cnt = int(pair_valid.sum())
    loss_delta = (np.square(d_l - d_t) * pair_valid).sum() / max(cnt, 1)

    # delta-of-delta on the compacted (valid-only, batch-major) delta lists
    L = B * (S - 1)
    pvf = pair_valid.reshape(-1)

    def dd(d_flat):
        dense = np.zeros(L, np.float64)
        dense[:cnt] = d_flat[pvf]
        prev = dense[:-1]
        den = np.where(prev != 0, prev, 1e-6)
        return (dense[1:] - prev) / den

    dd_l = dd(d_l.reshape(-1))
    dd_t = dd(d_t.reshape(-1))
    dd_valid = np.arange(L - 1) < (cnt - 1)
    n_dd = float(max(cnt - 1, 1))
    loss_dd = (np.square(dd_l - dd_t) * dd_valid).sum() / n_dd / 100.0

    return mse + loss_cos + loss_delta + loss_dd


def kernel(logits, tgt_out, tgt_padding_mask, _trace=False):
    logits = np.asarray(logits, dtype=np.float32)
    tgt_out = np.asarray(tgt_out, dtype=np.float32)
    mask = np.asarray(tgt_padding_mask).astype(bool)

    plan = _plan(mask)
    pos_b, pos_s, RL, K2, IG = plan
    by_pos, kres = _run_device(logits, tgt_out, plan, trace=_trace)

    # scatter position-ordered results back to full (B, S) arrays;
    # untouched positions stay 0 and are masked out in the finish.
    filled = pos_b >= 0
    rows = {}
    for q in ("ll", "tt", "lt"):
        f = np.zeros((B, S), np.float32)
        f[pos_b[filled], pos_s[filled]] = by_pos[q][filled]
        rows[q] = f
    # true pairs: position-adjacent, same batch, consecutive s. All of
    # them lie where the device computed a product (long region or the
    # even offset of an aligned pair).
    pair_ok = np.zeros(len(pos_b), bool)
    pair_ok[:-1] = ((pos_b[:-1] >= 0) & (pos_b[:-1] == pos_b[1:])
                    & (pos_s[:-1] + 1 == pos_s[1:]))
    gok = np.flatnonzero(pair_ok)
    for q in ("ln", "tn"):
        f = np.zeros((B, S), np.float32)
        f[pos_b[gok], pos_s[gok]] = by_pos[q][gok]
        rows[q] = f

    total = _finish_host(rows, mask)
    out = np.array(total, dtype=np.float32)
    if _trace:
        return out, kres
    return out


# revision 49
# speedup vs baseline: 1.1697x; 1.0167x over previous
"""Trainium2 Bass kernel for nn_Mixture_Loss_74053826118054.

Strategy (valid-row compaction + data parallel over 8 cores):
  Every term of the loss depends only on 5 per-(s,b)-row reductions over D:
    ll = sum_d l^2,  tt = sum_d t^2,  lt = sum_d l*t,
    ln = sum_d l[s]*l[s+1]  (consecutive sentences, same batch),
    tn = sum_d t[s]*t[s+1]
  The padding mask is known on the host, and every quantity is only ever
  USED on valid rows (~50%): masked MSE / cos / deltas all multiply by the
  valid mask, and ln/tn are only used on valid PAIRS (both rows valid,
  ~25%). The host packs only the valid rows and ships the packed array:
  half the HBM traffic and half the compute of the dense kernel.

  Rows are split into three regions by how much pair work they need:
  - "long region": rows of runs >= 3 consecutive valid, runs contiguous,
    chained across partitions with a +1 overlap slot; ln/tn computed for
    every slot adjacency (garbage at run boundaries, dropped on host).
  - "pair region": rows of length-2 runs, stored pair-aligned (first row
    in an even slot, second in the odd slot beside it). One ln/tn op per
    PAIR instead of per row — half the DVE pair work for these rows.
  - "iso region": isolated valid rows (no valid neighbor): ll/tt/lt only.
  Overflow rows spill into other regions' padding holes; pair validity
  is re-derived on the host from the position-aligned (b, s) map, so any
  garbage product (run boundaries, spilled rows, padding) is dropped
  exactly.

Device layout per core: the host builds the input SLOT-MAJOR
([slot, partition, {l,t}, D] - exactly the SBUF layout), so every chunk
load is one contiguous 1 MB DMA with one 8 KB descriptor per partition
(~400 GB/s observed vs ~275 GB/s for a strided layout). Per chunk: ACT
does the two squares with fused accumulate; DVE does lt/ln/tn as
scalar_tensor_tensor with fused accumulate. No GpSimd: its SBUF port is
shared with DVE ("POOL slot") and concurrent GpSimd ops double DVE
2-port op latency (measured: stt 2142 ns avg with GpSimd active vs
1135 ns clean). f16/bf16 data is NOT usable: the delta-of-delta term
amplifies input quantization into a ~25x loss error (measured); all
five reductions must be f32 exact.

DMA schedule: all chunk loads go back-to-back on the Sync engine's
HWDGE ring in consumption order, overlap slot last - a single ring
drains strictly FIFO, so each chunk gets the full wire and completes in
order (splitting across the two HWDGE rings measured ~2 us slower:
packet-granularity arbitration delays the early chunks). One semaphore
per chunk - completions are only FIFO per SDMA engine, so a summed
counter can be satisfied by a MIX of chunks (a race observed as
intermittent garbage). Each compute engine stores its own merged result
block (ACT: [ll|tt] on its ring right after its drain; DVE's [lt|ln|tn]
via Sync after dve_done). The tiny O(S*B) finish (cos, deltas,
rank-compaction, delta-of-delta) runs on host in float64, reproducing
the reference semantics exactly.

Timing on the fixed-seed data (RL=4, K2=1, IG=2, 9 MB/core): ~7 us NEFF
preamble (start barrier + instruction load, +-2 us run-to-run jitter),
~6 us first-two-chunk fill, ~20.6 us dense DVE (18 stt ops - the
compute floor; ACT is ~19.7 us), ~3.5 us store/epilogue tail. The wire
streams at ~2.5 us/MB fully overlapped under DVE.
"""

import numpy as np

from contextlib import ExitStack

import concourse.bass as bass
import concourse.mybir as mybir
from concourse.bass_utils import run_bass_kernel_spmd

F32 = mybir.dt.float32
AF = mybir.ActivationFunctionType
ALU = mybir.AluOpType

N_CORES = 8
S, B, D = 64, 256, 1024
P = 128

_cached = {}


def _build_program(RL, K2, IG):
    """RL chained long-run slots, K2 aligned pairs (2*K2 slots), IG iso
    slots, +1 overlap slot (loaded last)."""
    key = (RL, K2, IG)
    if key in _cached:
        return _cached[key]
    NC = RL + 2 * K2 + IG        # content slots = result columns
    NSLOT = NC + 1
    OV = NC                      # overlap slot index
    NLN = RL + K2                # ln/tn result columns
    nc = bass.Bass()
    # slot-major, exactly the SBUF layout: x[slot, partition] = [l | t]
    x_in = nc.dram_tensor("x", [NSLOT, P, 2, D], F32, kind="ExternalInput")
    out_act = nc.dram_tensor("act", [P, 2 * NC], F32, kind="ExternalOutput")
    out_dve = nc.dram_tensor("dve", [P, NC + 2 * NLN], F32,
                             kind="ExternalOutput")

    with ExitStack() as stack:
        ec = stack.enter_context
        # one semaphore per chunk: DMA completions are only FIFO per SDMA
        # engine, so a summed counter can pass with a mix of chunks
        csem = [ec(nc.semaphore(f"c{i}")) for i in range(NSLOT)]
        ovt = ec(nc.semaphore("ovt"))    # overlap t-half (l-half: csem[OV])
        out_sem = ec(nc.semaphore("out"))
        dve_done = ec(nc.semaphore("dvedone"))
        xbig = ec(nc.sbuf_tensor([P, NSLOT * 2 * D], F32))
        dummies = ec(nc.sbuf_tensor([P, 8], F32))
        ract = ec(nc.sbuf_tensor([P, 2 * NC], F32))       # [ll | tt]
        rdve = ec(nc.sbuf_tensor([P, NC + 2 * NLN], F32))  # [lt | ln | tn]
        block = ec(nc.Block())
        xc = xbig.ap().rearrange("p (c v d) -> p c v d", v=2, d=D)

        def chunk(slot, half):
            return xc[:, slot, half, :]

        def bcast(k):
            return dummies.ap()[:, k:k + 1].broadcast_to((P, D))

        rll = ract.ap()[:, 0:NC]
        rtt = ract.ap()[:, NC:2 * NC]
        rlt = rdve.ap()[:, 0:NC]
        rln = rdve.ap()[:, NC:NC + NLN]
        rtn = rdve.ap()[:, NC + NLN:NC + 2 * NLN]

        @block.sync
        def _(sync):
            # all loads on this one HWDGE ring, consumption order, overlap
            # slot last: a single ring drains strictly FIFO at full wire.
            # The overlap ships as separate l/t halves (own semaphores) so
            # the final ln starts one half-load before the final tn.
            for s in range(NC):
                sync.dma_start(out=xc[:, s, :, :],
                               in_=x_in[s]).then_inc(csem[s], 16)
            sync.dma_start(out=xc[:, OV, 0, :],
                           in_=x_in[OV, :, 0]).then_inc(csem[OV], 16)
            sync.dma_start(out=xc[:, OV, 1, :],
                           in_=x_in[OV, :, 1]).then_inc(ovt, 16)
            sync.wait_ge(dve_done, 1)
            sync.dma_start(out=out_dve[:], in_=rdve.ap()).then_inc(
                out_sem, 16)
            sync.wait_ge(out_sem, 32)

        @block.scalar
        def _(scalar):
            for col in range(NC):        # every content slot: two squares
                scalar.wait_ge(csem[col], 16)
                scalar.activation(bcast(0), chunk(col, 0), AF.Square,
                                  accum_out=rll[:, col:col + 1])
                scalar.activation(bcast(1), chunk(col, 1), AF.Square,
                                  accum_out=rtt[:, col:col + 1])
            scalar.drain()
            scalar.dma_start(out=out_act[:], in_=ract.ap()).then_inc(
                out_sem, 16)

        @block.vector
        def _(vector):

            def stt(i0, i1, acc):
                vector.scalar_tensor_tensor(
                    out=bcast(2), in0=i0, scalar=0.0, in1=i1,
                    op0=ALU.bypass, op1=ALU.mult, accum_out=acc)

            for j in range(RL):          # chained long-run slots
                vector.wait_ge(csem[j], 16)
                stt(chunk(j, 0), chunk(j, 1), rlt[:, j:j + 1])
                if j < RL - 1:           # j = RL-1 pairs with the overlap
                    vector.wait_ge(csem[j + 1], 16)   # slot, loaded last
                    stt(chunk(j, 0), chunk(j + 1, 0), rln[:, j:j + 1])
                    stt(chunk(j, 1), chunk(j + 1, 1), rtn[:, j:j + 1])
            for u in range(K2):          # aligned len-2 pairs
                sa, sb = RL + 2 * u, RL + 2 * u + 1
                vector.wait_ge(csem[sa], 16)
                stt(chunk(sa, 0), chunk(sa, 1), rlt[:, sa:sa + 1])
                vector.wait_ge(csem[sb], 16)
                stt(chunk(sb, 0), chunk(sb, 1), rlt[:, sb:sb + 1])
                stt(chunk(sa, 0), chunk(sb, 0), rln[:, RL + u:RL + u + 1])
                stt(chunk(sa, 1), chunk(sb, 1), rtn[:, RL + u:RL + u + 1])
            for k in range(IG):          # iso slots: lt only
                slot = RL + 2 * K2 + k
                vector.wait_ge(csem[slot], 16)
                stt(chunk(slot, 0), chunk(slot, 1), rlt[:, slot:slot + 1])
            vector.wait_ge(csem[OV], 16)
            stt(chunk(RL - 1, 0), chunk(OV, 0), rln[:, RL - 1:RL])
            vector.wait_ge(ovt, 16)
            stt(chunk(RL - 1, 1), chunk(OV, 1), rtn[:, RL - 1:RL])
            vector.drain().then_inc(dve_done, 1)

    _cached[key] = nc
    return nc


def _plan(mask):
    """Position-aligned packing plan from the padding mask.

    Positions: [long region | pair region | iso region], each a multiple
    of N_CORES*P. Returns (pos_b, pos_s, RL, K2, IG) with pos_b == -1 at
    padding. Long-run rows sit at positions [0, n_long) with runs
    contiguous; length-2 runs at pair positions PB+2q (first row) and
    PB+2q+1 (second); iso rows fill the iso region then any padding
    holes (their products are garbage the host drops via pair_ok).
    """
    valid = ~mask                                   # (B, S)
    vp = np.zeros((B, S + 3), bool)
    vp[:, 1:S + 1] = valid
    v, left = vp[:, 1:S + 1], vp[:, 0:S]
    r1, r2 = vp[:, 2:S + 2], vp[:, 3:S + 3]
    iso = v & ~left & ~r1
    start2 = v & ~left & r1 & ~r2                   # first row of len-2 run
    second2 = np.zeros_like(v)
    second2[:, 1:] = start2[:, :-1]
    longm = v & ~iso & ~start2 & ~second2           # rows of runs >= 3

    lb, ls = np.nonzero(longm)                      # lexicographic: runs
    fb, fs = np.nonzero(start2)                     # stay contiguous
    ib, is_ = np.nonzero(iso)
    n_long, n2, n_iso = len(lb), len(fb), len(ib)
    tot = n_long + 2 * n2 + n_iso

    CP = N_CORES * P
    RL = max(1, -(-n_long // CP))
    K2 = -(-n2 // CP)
    IG = max(0, -(-(tot - CP * RL - CP * 2 * K2) // CP))
    PB = CP * RL                                    # pair region base
    IB = PB + CP * 2 * K2                           # iso region base
    NPOS = IB + CP * IG

    pos_b = np.full(NPOS, -1, np.int64)
    pos_s = np.zeros(NPOS, np.int64)
    pos_b[:n_long] = lb
    pos_s[:n_long] = ls
    if n2:
        pos_b[PB:PB + 2 * n2:2] = fb
        pos_s[PB:PB + 2 * n2:2] = fs
        pos_b[PB + 1:PB + 2 * n2:2] = fb
        pos_s[PB + 1:PB + 2 * n2:2] = fs + 1
    # iso rows: iso region first, then spill into padding holes
    holes = np.concatenate([np.arange(IB, NPOS),
                            np.arange(n_long, PB),
                            np.arange(PB + 2 * n2, IB)])
    pos_b[holes[:n_iso]] = ib
    pos_s[holes[:n_iso]] = is_
    return pos_b, pos_s, RL, K2, IG


def _run_device(logits, tgt_out, plan, trace=False):
    pos_b, pos_s, RL, K2, IG = plan
    nc = _build_program(RL, K2, IG)
    NC = RL + 2 * K2 + IG
    NLN = RL + K2
    NPOS = len(pos_b)
    PB = N_CORES * P * RL
    IB = PB + N_CORES * P * 2 * K2

    lf = np.swapaxes(logits, 0, 1)                  # (B, S, D) view
    tf = np.swapaxes(tgt_out, 0, 1)
    data = np.zeros((NPOS + 1, 2, D), np.float32)   # position-ordered rows
    filled = pos_b >= 0
    data[:NPOS][filled, 0] = lf[pos_b[filled], pos_s[filled]]
    data[:NPOS][filled, 1] = tf[pos_b[filled], pos_s[filled]]

    in_maps = []
    for c in range(N_CORES):
        x = np.empty((NC + 1, P, 2, D), np.float32)
        for base, width, lo in ((0, RL, 0), (PB, 2 * K2, RL),
                                (IB, IG, RL + 2 * K2)):
            if width == 0:
                continue
            blk = data[base + c * P * width:base + (c + 1) * P * width]
            x[lo:lo + width] = blk.reshape(P, width, 2, D).transpose(
                1, 0, 2, 3)
        # overlap slot: next partition's (or next core's) first long row
        x[NC] = data[c * P * RL + RL:(c + 1) * P * RL + RL:RL]
        in_maps.append({"x": x})
    kres = run_bass_kernel_spmd(nc, in_maps, list(range(N_CORES)),
                                trace=trace)

    # device columns -> position-ordered arrays [long | pair | iso]
    res = kres.results
    by_pos = {}
    for q, grab in (("ll", lambda a: a["act"][:, :NC]),
                    ("tt", lambda a: a["act"][:, NC:]),
                    ("lt", lambda a: a["dve"][:, :NC])):
        parts = []
        for lo, width in ((0, RL), (RL, 2 * K2), (RL + 2 * K2, IG)):
            if width:
                parts.append(np.concatenate(
                    [grab(res[c])[:, lo:lo + width].reshape(P * width)
                     for c in range(N_CORES)]))
        by_pos[q] = np.concatenate(parts)
    # ln/tn: cols [0, RL) = long adjacencies, cols [RL, RL+K2) = pairs
    for q, off in (("ln", NC), ("tn", NC + NLN)):
        lng = np.concatenate(
            [res[c]["dve"][:, off:off + RL].reshape(P * RL)
             for c in range(N_CORES)])
        f = np.zeros(NPOS, np.float32)
        f[:PB] = lng                    # value at pos g = product(g, g+1)
        if K2:
            pair = np.concatenate(
                [res[c]["dve"][:, off + RL:off + NLN].reshape(P * K2)
                 for c in range(N_CORES)])
            f[PB:IB:2] = pair
        by_pos[q] = f
    return by_pos, kres


def _finish_host(rows, mask):
    """Host-side float64 finish: reproduce reference semantics exactly."""
    ll = rows["ll"].astype(np.float64)
    tt = rows["tt"].astype(np.float64)
    lt = rows["lt"].astype(np.float64)
    ln = rows["ln"].astype(np.float64)
    tn = rows["tn"].astype(np.float64)

    valid = ~mask                     # (B, S)
    n_valid = float(valid.sum())

    # masked MSE: sum over valid rows of sum_d (l-t)^2 = ll - 2lt + tt
    mse = ((ll - 2.0 * lt + tt) * valid).sum() / (n_valid * D)

    # CosineEmbeddingLoss part (eps = 1e-8)
    na = np.maximum(np.sqrt(ll), 1e-8)
    nb = np.maximum(np.sqrt(tt), 1e-8)
    c = lt / (na * nb)
    loss_cos = ((1.0 - c) * valid).sum() / n_valid

    # consecutive-sentence cosine deltas (eps = 1e-6), shape (B, S-1)
    nl = np.maximum(np.sqrt(ll), 1e-6)
    nt = np.maximum(np.sqrt(tt), 1e-6)
    d_l = ln[:, :S - 1] / (nl[:, :-1] * nl[:, 1:])
    d_t = tn[:, :S - 1] / (nt[:, :-1] * nt[:, 1:])
    pair_valid = valid[:, :-1] & valid[:, 1:]
    cnt = int(pair_valid.sum())
    loss_delta = (np.square(d_l - d_t) * pair_valid).sum() / max(cnt, 1)

    # delta-of-delta on the compacted (valid-only, batch-major) delta lists
    L = B * (S - 1)
    pvf = pair_valid.reshape(-1)

    def dd(d_flat):
        dense = np.zeros(L, np.float64)
        dense[:cnt] = d_flat[pvf]
        prev = dense[:-1]
        den = np.where(prev != 0, prev, 1e-6)
        return (dense[1:] - prev) / den

    dd_l = dd(d_l.reshape(-1))
    dd_t = dd(d_t.reshape(-1))
    dd_valid = np.arange(L - 1) < (cnt - 1)
    n_dd = float(max(cnt - 1, 1))
    loss_dd = (np.square(dd_l - dd_t) * dd_valid).sum() / n_dd / 100.0

    return mse + loss_cos + loss_delta + loss_dd


def kernel(logits, tgt_out, tgt_padding_mask, _trace=False):
    logits = np.asarray(logits, dtype=np.float32)
    tgt_out = np.asarray(tgt_out, dtype=np.float32)
    mask = np.asarray(tgt_padding_mask).astype(bool)

    plan = _plan(mask)
    pos_b, pos_s, RL, K2, IG = plan
    by_pos, kres = _run_device(logits, tgt_out, plan, trace=_trace)

    # scatter position-ordered results back to full (B, S) arrays;
    # untouched positions stay 0 and are masked out in the finish.
    filled = pos_b >= 0
    rows = {}
    for q in ("ll", "tt", "lt"):
        f = np.zeros((B, S), np.float32)
        f[pos_b[filled], pos_s[filled]] = by_pos[q][filled]
        rows[q] = f
    # true pairs: position-adjacent, same batch, consecutive s. All of
    # them lie where the device computed a product (long region or the
    # even offset of an aligned pair).
    pair_ok = np.zeros(len(pos_b), bool)
    pair_ok[:-1] = ((pos_b[:-1] >= 0) & (pos_b[:-1] == pos_b[1:])
                    & (pos_s[:-1] + 1 == pos_s[1:]))
    gok = np.flatnonzero(pair_ok)
    for q in ("ln", "tn"):
        f = np.zeros((B, S), np.float32)
        f[pos_b[gok], pos_s[gok]] = by_pos[q][gok]
        rows[q] = f

    total = _finish_host(rows, mask)
    out = np.array(total, dtype=np.float32)
    if _trace:
        return out, kres
    return out
